# revision 49
# baseline (speedup 1.0000x reference)
"""Single-head full-attention layer on 8 Trainium2 NeuronCores (fp8 DoubleRow).

reference:
    q = seq @ Wq; k = seq @ Wk; v = seq @ Wv          # [B,S,D], D=1024
    scores = q @ k.T / sqrt(D)                        # [B,S,S]
    out = seq + softmax(scores) @ v * mask            # [B,S,D]

Sharding: 8 cores = 4 batches x 2 sequence-halves, each core owning 1024
tokens.  The K projection never runs on device: the host folds
M = Wq @ Wk^T (scores = seq @ M @ seq_full^T), so the only cross-core
data needed for scores is the RAW transposed input sequence -- which is
an input tensor with zero compute dependency.  Its pair-AllGather is
issued as the kernel's first instruction and completes long before the
scores phase consumes the peer half.  V is projected locally for own
keys and exchanged via a second AllGather that hides under Q'/scores.

Attention runs in a per-core "local-first" key order [own 1024 keys,
peer 1024 keys] (softmax+PV are key-permutation invariant), keeping the
SPMD program identical on all cores; the peer halves of both AllGather
outputs are read with dynamic (register-offset) DMA slices, the slot
index coming from a per-core host input.

All matmuls run in fp8(E4M3) with perf_mode=DoubleRow (two contraction
rows per PE cell): operands live in 3D SBUF tiles [128, ksub, free] and
each matmul consumes a [:, k:k+2, :] slice. Numerics:
  - host scales M and Wv by 32 before the fp8 cast; seq is cast raw
    (N(0,1) fits fp8).  Q' = seq @ M lands at 32x, exactly like the old
    q, so exp's scale is 2^-10 (1/sqrt(D) * 1/32).
  - exp is shifted by -3 so attn values stay below fp8e4's +-240 max
    (softmax is shift-invariant).
  - scores/colsum/out accumulate in fp32 PSUM; the last matmul emits
    the output in [q, d] layout so 1/colsum is a per-partition scalar
    and normalize + residual-add (bf16 residual) fuse into one
    scalar_tensor_tensor.  The output mask is folded into Wv on the
    host.  The colsum PSUM tile is [2, 512] (one bank) so the matmul
    pool can hold 7 banks.
"""

import numpy as np
import ml_dtypes

import concourse.bass as bass
import concourse.mybir as mybir
import concourse.tile as tile
from concourse import bacc, bass_utils

B, S, D = 4, 2048, 1024
N_CORES = 8
SH = S // 2          # queries / own keys per core
PD = 128             # partition dim
KD = D // PD         # 8 ksub chunks over d
KH = SH // PD        # 8 ksub chunks over own keys
KC = S // PD         # 16 ksub chunks over all keys
NT = 512             # matmul free-dim tile (one PSUM bank of fp32)
F8 = mybir.dt.float8e4
F32 = mybir.dt.float32
BF16 = mybir.dt.bfloat16
W_SCALE = 32.0
EXP_SCALE = 1.0 / (32.0 * W_SCALE)     # 1/sqrt(D) / W_SCALE
EXP_SHIFT = -3.0
DR = mybir.MatmulPerfMode.DoubleRow
WARMUP = 180

_FP8 = ml_dtypes.float8_e4m3
_GROUPS = [[0, 1], [2, 3], [4, 5], [6, 7]]


def _build_kernel(tc):
    nc = tc.nc
    seqTq = nc.dram_tensor("seqTq", [D, SH], F8, kind="ExternalInput").ap()
    m3w = nc.dram_tensor("m3w", [D, D], F8, kind="ExternalInput").ap()
    wv = nc.dram_tensor("wv", [D, D], F8, kind="ExternalInput").ap()
    seqTh = nc.dram_tensor("seqTh", [SH, D], BF16, kind="ExternalInput").ap()
    peer_t = nc.dram_tensor("peer", [1, 1], mybir.dt.uint32, kind="ExternalInput")
    outT = nc.dram_tensor("outT", [SH, D], BF16, kind="ExternalOutput").ap()

    Exp = mybir.ActivationFunctionType.Exp

    with (
        tc.tile_pool(name="p_seq", bufs=1) as p_seq,
        tc.tile_pool(name="p_sp", bufs=1) as p_sp,
        tc.tile_pool(name="p_w", bufs=2) as p_w,
        tc.tile_pool(name="p_qt", bufs=1) as p_qt,
        tc.tile_pool(name="p_vo", bufs=1) as p_vo,
        tc.tile_pool(name="p_vp", bufs=1) as p_vp,
        tc.tile_pool(name="p_at", bufs=1) as p_at,
        tc.tile_pool(name="p_sh", bufs=1) as p_sh,
        tc.tile_pool(name="p_o", bufs=4) as p_o,
        tc.tile_pool(name="p_msc", bufs=1) as p_msc,
        tc.tile_pool(name="p_dram", bufs=1, space="DRAM") as p_dram,
        tc.tile_pool(name="p_mm", bufs=6, space="PSUM") as p_mm,
        tc.tile_pool(name="p_cs", bufs=1, space="PSUM") as p_cs,
    ):
        # ---- seq exchange: pure input data, no compute dependency -> the
        # AllGather is gated only by one DRAM->DRAM bounce of the input
        # (collectives cannot read IO tensors) and its latency hides under
        # warmup + V_own + Q' + scores-own entirely.
        ib_x = p_dram.tile([D, SH], F8, tag="ibx", name="ib_x")
        ob_x = p_dram.tile([2, D, SH], F8, tag="obx", name="ob_x")

        # ---- resident inputs.  The V phase's k-loop consumes (seqq, wv)
        # j-chunks in ascending order, so loading each tensor as two
        # interleaved half-chunks lets the first V matmuls start once 1MB
        # has landed instead of waiting for the full 2MB.  The collective
        # input bounce follows them -- its only deadline is the ~40us ncfw
        # stream-init floor, which it beats by a mile.
        HD = D // 2
        seqq3 = p_seq.tile([PD, KD, SH], F8, tag="seqq", name="seqq3")
        wv3 = p_w.tile([PD, KD, D], F8, tag="wv", name="wv3")
        nc.sync.dma_start(
            seqq3[:, 0:KD // 2, :],
            seqTq[0:HD, :].rearrange("(j p) s -> p j s", p=PD))
        nc.sync.dma_start(
            wv3[:, 0:KD // 2, :],
            wv[0:HD, :].rearrange("(j p) d -> p j d", p=PD))
        nc.sync.dma_start(
            seqq3[:, KD // 2:KD, :],
            seqTq[HD:D, :].rearrange("(j p) s -> p j s", p=PD))
        nc.sync.dma_start(
            wv3[:, KD // 2:KD, :],
            wv[HD:D, :].rearrange("(j p) d -> p j d", p=PD))
        nc.sync.dma_start(ib_x[:], seqTq)
        nc.gpsimd.collective_compute(
            "AllGather", mybir.AluOpType.bypass, replica_groups=_GROUPS,
            ins=[ib_x.opt()], outs=[ob_x.opt()],
        )
        m3 = p_w.tile([PD, KD, D], F8, tag="m3", name="m3")
        nc.sync.dma_start(m3[:], m3w.rearrange("(j p) d -> p j d", p=PD))
        # residual input (bf16)
        sh3 = p_sh.tile([PD, KH, D], BF16, tag="sh", name="sh3")
        nc.sync.dma_start(sh3[:], seqTh.rearrange("(j p) d -> p j d", p=PD))

        # V is exchanged as two half-size AllGathers so the first half's
        # data (queued on the cc stream behind the seq exchange) lands
        # before the O phase touches the peer V.
        ib_v = p_dram.tile([SH, D], F8, tag="ibv", name="ib_v")
        ob_v1 = p_dram.tile([2, SH // 2, D], F8, tag="obv1", name="ob_v1")
        ob_v2 = p_dram.tile([2, SH // 2, D], F8, tag="obv2", name="ob_v2")

        # ---- HAM warm-up: keep the PE busy on dummy matmuls during the
        # input-DMA wait so the clock gate is at 2.4GHz when real work starts
        wu_sb = p_msc.tile([PD, 2, 16], F8, tag="wu", name="wu_sb")
        nc.vector.memset(wu_sb[:], 0.0)
        ps_wu = p_mm.tile([PD, 16], F32, tag="mm", name="ps_wu")
        for i in range(WARMUP):
            nc.tensor.matmul(
                ps_wu[0:16, 0:16], wu_sb[:, 0:2, 0:16], wu_sb[:, 0:2, 0:16],
                start=(i == 0), stop=(i == WARMUP - 1), perf_mode=DR,
            )

        # ---- V_own = seq_own @ (Wv * mask), bounce out, AllGather ----------
        vo3 = p_vo.tile([PD, KH, D], F8, tag="vo", name="vo3")
        for m in range(KH):
            for n in range(D // NT):
                ps = p_mm.tile([PD, NT], F32, tag="mm", name=f"ps_v{m}_{n}")
                for k in range(0, KD, 2):
                    nc.tensor.matmul(
                        ps[:],
                        seqq3[:, k:k + 2, m * PD:(m + 1) * PD],
                        wv3[:, k:k + 2, n * NT:(n + 1) * NT],
                        start=(k == 0),
                        stop=(k == KD - 2),
                        perf_mode=DR,
                    )
                nc.vector.tensor_copy(vo3[:, m, n * NT:(n + 1) * NT], ps[:])
            nc.sync.dma_start(ib_v[m * PD:(m + 1) * PD, :], vo3[:, m, :])
            if m == KH // 2 - 1:
                nc.gpsimd.collective_compute(
                    "AllGather", mybir.AluOpType.bypass, replica_groups=_GROUPS,
                    ins=[ib_v[0:SH // 2, :]], outs=[ob_v1.opt()],
                )
        nc.gpsimd.collective_compute(
            "AllGather", mybir.AluOpType.bypass, replica_groups=_GROUPS,
            ins=[ib_v[SH // 2:SH, :]], outs=[ob_v2.opt()],
        )

        # ---- Q'^T = (seq_own @ M).T with M = Wq Wk^T folded on the host ----
        qt3 = p_qt.tile([PD, KD, SH], F8, tag="qt", name="qt3")
        for m in range(KD):
            for n in range(SH // NT):
                ps = p_mm.tile([PD, NT], F32, tag="mm", name=f"ps_q{m}_{n}")
                for k in range(0, KD, 2):
                    nc.tensor.matmul(
                        ps[:],
                        m3[:, k:k + 2, m * PD:(m + 1) * PD],
                        seqq3[:, k:k + 2, n * NT:(n + 1) * NT],
                        start=(k == 0),
                        stop=(k == KD - 2),
                        perf_mode=DR,
                    )
                nc.vector.tensor_copy(qt3[:, m, n * NT:(n + 1) * NT], ps[:])

        # ---- peer halves of the exchanges, via dynamic-slot DMA slices.
        # peer_slot (0|1) is a per-core host input; the sync engine blocks
        # here until each AllGather lands, but everything else it must issue
        # before the O phase (inputs, ib_v bounces, sh3) is already in flight.
        preg = nc.sync.alloc_register("peer_slot")
        nc.sync.reg_load(preg, peer_t[0:1, 0:1])
        pslot = nc.sync.snap(preg, donate=True, min_val=0, max_val=1)
        # batched gathers: one rearranged DMA per collective output rather
        # than one per 128-row chunk (each sync issue costs ~0.6us, paid
        # right after the collective completes -- on the critical path)
        seqp3 = p_sp.tile([PD, KD, SH], F8, tag="sp", name="seqp3")
        nc.sync.dma_start(
            seqp3[:],
            ob_x[bass.ds(pslot, 1), :, :].rearrange(
                "o (j p) s -> (o p) j s", p=PD),
        )
        v_other = p_vp.tile([PD, KH, D], F8, tag="vp", name="v_other")
        nc.sync.dma_start(
            v_other[:, 0:KH // 2, :],
            ob_v1[bass.ds(pslot, 1), :, :].rearrange(
                "o (j p) d -> (o p) j d", p=PD),
        )
        nc.sync.dma_start(
            v_other[:, KH // 2:KH, :],
            ob_v2[bass.ds(pslot, 1), :, :].rearrange(
                "o (j p) d -> (o p) j d", p=PD),
        )

        # ---- scoresT -> exp(shifted) -> colsum, local-first key order ------
        # 32.0 (exact in fp8) folds V's W_SCALE into the colsum so the
        # reciprocal needs no extra rescale
        ones3 = p_msc.tile([PD, 2, 16], F8, tag="ones", name="ones3")
        nc.vector.memset(ones3[:], float(W_SCALE))
        ebias = p_msc.tile([PD, 1], F32, tag="ebias", name="ebias")
        nc.vector.memset(ebias[:], EXP_SHIFT)
        cs_ps = p_cs.tile([1, SH], F32, tag="cs", name="cs")
        at3 = p_at.tile([PD, KC, SH], F8, tag="at", name="at3")

        def colsum_mm(m):
            for n in range(SH // NT):
                nc.tensor.matmul(
                    cs_ps[:, n * NT:(n + 1) * NT],
                    ones3[:, 0:2, 0:1],
                    at3[:, m:m + 2, n * NT:(n + 1) * NT],
                    start=(m == 0),
                    stop=(m == KC - 2),
                    perf_mode=DR,
                )

        # key chunks 0..7 score against seqq3 (own, no exchange wait),
        # 8..15 against the peer half; the AllGather hides under 0..7.
        for m in range(KC):
            sq3 = seqq3 if m < KD else seqp3
            mm_ = m if m < KD else m - KD
            for n in range(SH // NT):
                ps = p_mm.tile([PD, NT], F32, tag="mm", name=f"ps_s{m}_{n}")
                for k in range(0, KD, 2):
                    nc.tensor.matmul(
                        ps[:],
                        sq3[:, k:k + 2, mm_ * PD:(mm_ + 1) * PD],
                        qt3[:, k:k + 2, n * NT:(n + 1) * NT],
                        start=(k == 0),
                        stop=(k == KD - 2),
                        perf_mode=DR,
                    )
                nc.scalar.activation(
                    at3[:, m, n * NT:(n + 1) * NT], ps[:], Exp,
                    bias=ebias[:], scale=EXP_SCALE,
                )
            # colsum pairs ksubs (m, m+1); emit one pair late so the PE
            # never waits on ACT's exp
            if m >= 3 and m % 2 == 1:
                colsum_mm(m - 3)
        colsum_mm(KC - 2)

        # ---- 1/(32*colsum) transposed to per-partition [128, 8] -------------
        # reciprocal on [1, SH] runs on one DVE lane (6.5us); transpose the
        # colsum to [128, 8] via a DRAM bounce first so it takes ~0.2us.
        cs_sb = p_msc.tile([1, SH], F32, tag="cs_sb", name="cs_sb")
        nc.vector.tensor_copy(cs_sb[:], cs_ps[:])
        cs_d = p_dram.tile([1, SH], F32, tag="csd", name="cs_d")
        nc.gpsimd.dma_start(cs_d[:], cs_sb[:])
        csT = p_msc.tile([PD, KH], F32, tag="csT", name="csT")
        nc.gpsimd.dma_start(csT[:, :], cs_d.rearrange("o (m p) -> (o p) m", p=PD))
        recipT = p_msc.tile([PD, KH], F32, tag="recipT", name="recipT")
        nc.vector.reciprocal(recipT[:, :], csT[:, :])

        # ---- O = AT.T @ V in [q, d] layout; fused normalize + residual ------
        # out[q, d] = (sum_key at[key, q] * v[key, d]) * recip[q] + seq[q, d]
        # key chunk pairs 0..7 stream from vo3 (own V), 8..15 from v_other.
        for m in range(KH):
            o_t = p_o.tile([PD, D], BF16, tag="o", name=f"o{m}")
            for n in range(D // NT):
                ps = p_mm.tile([PD, NT], F32, tag="mm", name=f"ps_o{m}_{n}")
                for k in range(0, KC, 2):
                    v3t = vo3 if k < KH else v_other
                    kk = k if k < KH else k - KH
                    nc.tensor.matmul(
                        ps[:],
                        at3[:, k:k + 2, m * PD:(m + 1) * PD],
                        v3t[:, kk:kk + 2, n * NT:(n + 1) * NT],
                        start=(k == 0),
                        stop=(k == KC - 2),
                        perf_mode=DR,
                    )
                nc.vector.scalar_tensor_tensor(
                    o_t[:, n * NT:(n + 1) * NT],
                    ps[:],
                    recipT[:, m:m + 1],
                    sh3[:, m, n * NT:(n + 1) * NT],
                    op0=mybir.AluOpType.mult,
                    op1=mybir.AluOpType.add,
                )
            nc.sync.dma_start(outT[m * PD:(m + 1) * PD, :], o_t[:])


_NC_CACHE = None


def _get_nc():
    global _NC_CACHE
    if _NC_CACHE is None:
        nc = bacc.Bacc(
            "TRN2", target_bir_lowering=False, debug=False, num_devices=N_CORES
        )
        with tile.TileContext(nc) as tc:
            _build_kernel(tc)
        nc.compile()
        _NC_CACHE = nc
    return _NC_CACHE


def _prep_in_maps(seq, Wq, Wk, Wv, mask):
    seq = np.asarray(seq, dtype=np.float32)
    M = np.asarray(Wq, dtype=np.float32) @ np.asarray(Wk, dtype=np.float32).T
    m_f8 = (M * W_SCALE).astype(_FP8)
    wvm_f8 = (np.asarray(Wv, dtype=np.float32)
              * np.asarray(mask, dtype=np.float32)[None, :] * W_SCALE).astype(_FP8)
    in_maps = []
    for c in range(N_CORES):
        b, h = divmod(c, 2)
        seqT_own = np.ascontiguousarray(seq[b, h * SH:(h + 1) * SH, :].T)  # [D, SH]
        in_maps.append({
            "seqTq": seqT_own.astype(_FP8),
            "m3w": m_f8,
            "wv": wvm_f8,
            "seqTh": np.ascontiguousarray(
                seq[b, h * SH:(h + 1) * SH, :]).astype(ml_dtypes.bfloat16),
            "peer": np.array([[1 - h]], dtype=np.uint32),
        })
    return in_maps


def _run(seq, Wq, Wk, Wv, mask, trace=False, **run_kwargs):
    nc = _get_nc()
    in_maps = _prep_in_maps(seq, Wq, Wk, Wv, mask)
    res = bass_utils.run_bass_kernel_spmd(
        nc, in_maps, core_ids=list(range(N_CORES)), trace=trace, **run_kwargs
    )
    out = np.empty((B, S, D), dtype=np.float32)
    for c in range(N_CORES):
        b, h = divmod(c, 2)
        out[b, h * SH:(h + 1) * SH, :] = np.asarray(
            res.results[c]["outT"]).astype(np.float32)
    return out, res


def kernel(seq, Wq, Wk, Wv, mask):
    out, _ = _run(seq, Wq, Wk, Wv, mask)
    return out


# revision 51
# speedup vs baseline: 1.0807x; 1.0807x over previous
"""Single-head full-attention layer on 8 Trainium2 NeuronCores (fp8 DoubleRow).

reference:
    q = seq @ Wq; k = seq @ Wk; v = seq @ Wv          # [B,S,D], D=1024
    scores = q @ k.T / sqrt(D)                        # [B,S,S]
    out = seq + softmax(scores) @ v * mask            # [B,S,D]

Sharding: 8 cores = 4 batches x 2 sequence-halves, each core owning 1024
tokens.  The K projection never runs on device: the host folds
M = Wq @ Wk^T (scores = seq @ M @ seq_full^T), so the only cross-core
data needed for scores is the RAW transposed input sequence -- which is
an input tensor with zero compute dependency.  Its pair-AllGather is
issued as the kernel's first instruction and completes long before the
scores phase consumes the peer half.  V is projected locally for own
keys and exchanged via a second AllGather that hides under Q'/scores.

Attention runs in a per-core "local-first" key order [own 1024 keys,
peer 1024 keys] (softmax+PV are key-permutation invariant), keeping the
SPMD program identical on all cores; the peer halves of both AllGather
outputs are read with dynamic (register-offset) DMA slices, the slot
index coming from a per-core host input.

All matmuls run in fp8(E4M3) with perf_mode=DoubleRow (two contraction
rows per PE cell): operands live in 3D SBUF tiles [128, ksub, free] and
each matmul consumes a [:, k:k+2, :] slice. Numerics:
  - host scales M and Wv by 32 before the fp8 cast; seq is cast raw
    (N(0,1) fits fp8).  Q' = seq @ M lands at 32x, exactly like the old
    q, so exp's scale is 2^-10 (1/sqrt(D) * 1/32).
  - exp is shifted by -3 so attn values stay below fp8e4's +-240 max
    (softmax is shift-invariant).
  - scores/colsum/out accumulate in fp32 PSUM; the last matmul emits
    the output in [q, d] layout so 1/colsum is a per-partition scalar
    and normalize + residual-add (bf16 residual) fuse into one
    scalar_tensor_tensor.  The output mask is folded into Wv on the
    host.  The colsum PSUM tile is [2, 512] (one bank) so the matmul
    pool can hold 7 banks.
"""

import numpy as np
import ml_dtypes

import concourse.bass as bass
import concourse.mybir as mybir
import concourse.tile as tile
from concourse import bacc, bass_utils

B, S, D = 4, 2048, 1024
N_CORES = 8
SH = S // 2          # queries / own keys per core
PD = 128             # partition dim
KD = D // PD         # 8 ksub chunks over d
KH = SH // PD        # 8 ksub chunks over own keys
KC = S // PD         # 16 ksub chunks over all keys
NT = 512             # matmul free-dim tile (one PSUM bank of fp32)
F8 = mybir.dt.float8e4
F32 = mybir.dt.float32
BF16 = mybir.dt.bfloat16
W_SCALE = 32.0
EXP_SCALE = 1.0 / (32.0 * W_SCALE)     # 1/sqrt(D) / W_SCALE
EXP_SHIFT = -3.0
DR = mybir.MatmulPerfMode.DoubleRow
WARMUP = 190

_FP8 = ml_dtypes.float8_e4m3
_GROUPS = [[0, 1], [2, 3], [4, 5], [6, 7]]


def _build_kernel(tc):
    nc = tc.nc
    seqTq = nc.dram_tensor("seqTq", [D, SH], F8, kind="ExternalInput").ap()
    m3w = nc.dram_tensor("m3w", [D, D], F8, kind="ExternalInput").ap()
    wv = nc.dram_tensor("wv", [D, D], F8, kind="ExternalInput").ap()
    seqTh = nc.dram_tensor("seqTh", [SH, D], BF16, kind="ExternalInput").ap()
    peer_t = nc.dram_tensor("peer", [1, 1], mybir.dt.uint32, kind="ExternalInput")
    outT = nc.dram_tensor("outT", [SH, D], BF16, kind="ExternalOutput").ap()

    Exp = mybir.ActivationFunctionType.Exp

    with (
        tc.tile_pool(name="p_seq", bufs=1) as p_seq,
        tc.tile_pool(name="p_sp", bufs=1) as p_sp,
        tc.tile_pool(name="p_w", bufs=2) as p_w,
        tc.tile_pool(name="p_qt", bufs=1) as p_qt,
        tc.tile_pool(name="p_vo", bufs=1) as p_vo,
        tc.tile_pool(name="p_vp", bufs=1) as p_vp,
        tc.tile_pool(name="p_at", bufs=1) as p_at,
        tc.tile_pool(name="p_sh", bufs=1) as p_sh,
        tc.tile_pool(name="p_o", bufs=4) as p_o,
        tc.tile_pool(name="p_msc", bufs=1) as p_msc,
        tc.tile_pool(name="p_dram", bufs=1, space="DRAM") as p_dram,
        tc.tile_pool(name="p_mm", bufs=6, space="PSUM") as p_mm,
        tc.tile_pool(name="p_cs", bufs=1, space="PSUM") as p_cs,
    ):
        # ---- seq exchange: pure input data, no compute dependency -> the
        # AllGather is gated only by one DRAM->DRAM bounce of the input
        # (collectives cannot read IO tensors) and its latency hides under
        # warmup + V_own + Q' + scores-own entirely.
        ib_x = p_dram.tile([D, SH], F8, tag="ibx", name="ib_x")
        ob_x = p_dram.tile([2, D, SH], F8, tag="obx", name="ob_x")

        # ---- resident inputs.  The V phase's k-loop consumes (seqq, wv)
        # j-chunks in ascending order, so loading each tensor as two
        # interleaved half-chunks lets the first V matmuls start once 1MB
        # has landed instead of waiting for the full 2MB.  The collective
        # input bounce follows them -- its only deadline is the ~40us ncfw
        # stream-init floor, which it beats by a mile.
        HD = D // 2
        seqq3 = p_seq.tile([PD, KD, SH], F8, tag="seqq", name="seqq3")
        wv3 = p_w.tile([PD, KD, D], F8, tag="wv", name="wv3")
        nc.sync.dma_start(
            seqq3[:, 0:KD // 2, :],
            seqTq[0:HD, :].rearrange("(j p) s -> p j s", p=PD))
        nc.sync.dma_start(
            wv3[:, 0:KD // 2, :],
            wv[0:HD, :].rearrange("(j p) d -> p j d", p=PD))
        nc.sync.dma_start(
            seqq3[:, KD // 2:KD, :],
            seqTq[HD:D, :].rearrange("(j p) s -> p j s", p=PD))
        nc.sync.dma_start(
            wv3[:, KD // 2:KD, :],
            wv[HD:D, :].rearrange("(j p) d -> p j d", p=PD))
        nc.sync.dma_start(ib_x[:], seqTq)
        nc.gpsimd.collective_compute(
            "AllGather", mybir.AluOpType.bypass, replica_groups=_GROUPS,
            ins=[ib_x.opt()], outs=[ob_x.opt()],
        )
        m3 = p_w.tile([PD, KD, D], F8, tag="m3", name="m3")
        nc.sync.dma_start(m3[:], m3w.rearrange("(j p) d -> p j d", p=PD))
        # residual input (bf16)
        sh3 = p_sh.tile([PD, KH, D], BF16, tag="sh", name="sh3")
        nc.sync.dma_start(sh3[:], seqTh.rearrange("(j p) d -> p j d", p=PD))

        # V is exchanged as two half-size AllGathers so the first half's
        # data (queued on the cc stream behind the seq exchange) lands
        # before the O phase touches the peer V.
        ib_v = p_dram.tile([SH, D], F8, tag="ibv", name="ib_v")
        ob_v1 = p_dram.tile([2, SH // 2, D], F8, tag="obv1", name="ob_v1")
        ob_v2 = p_dram.tile([2, SH // 2, D], F8, tag="obv2", name="ob_v2")

        # ---- HAM warm-up: keep the PE busy on dummy matmuls during the
        # input-DMA wait so the clock gate is at 2.4GHz when real work starts
        wu_sb = p_msc.tile([PD, 2, 16], F8, tag="wu", name="wu_sb")
        nc.vector.memset(wu_sb[:], 0.0)
        ps_wu = p_mm.tile([PD, 16], F32, tag="mm", name="ps_wu")
        for i in range(WARMUP):
            nc.tensor.matmul(
                ps_wu[0:16, 0:16], wu_sb[:, 0:2, 0:16], wu_sb[:, 0:2, 0:16],
                start=(i == 0), stop=(i == WARMUP - 1), perf_mode=DR,
            )

        # ---- V_own = seq_own @ (Wv * mask), bounce out, AllGather ----------
        vo3 = p_vo.tile([PD, KH, D], F8, tag="vo", name="vo3")
        for m in range(KH):
            for n in range(D // NT):
                ps = p_mm.tile([PD, NT], F32, tag="mm", name=f"ps_v{m}_{n}")
                for k in range(0, KD, 2):
                    nc.tensor.matmul(
                        ps[:],
                        seqq3[:, k:k + 2, m * PD:(m + 1) * PD],
                        wv3[:, k:k + 2, n * NT:(n + 1) * NT],
                        start=(k == 0),
                        stop=(k == KD - 2),
                        perf_mode=DR,
                    )
                nc.vector.tensor_copy(vo3[:, m, n * NT:(n + 1) * NT], ps[:])
            nc.sync.dma_start(ib_v[m * PD:(m + 1) * PD, :], vo3[:, m, :])
            if m == KH // 2 - 1:
                nc.gpsimd.collective_compute(
                    "AllGather", mybir.AluOpType.bypass, replica_groups=_GROUPS,
                    ins=[ib_v[0:SH // 2, :]], outs=[ob_v1.opt()],
                )
        nc.gpsimd.collective_compute(
            "AllGather", mybir.AluOpType.bypass, replica_groups=_GROUPS,
            ins=[ib_v[SH // 2:SH, :]], outs=[ob_v2.opt()],
        )

        # ---- Q'^T = (seq_own @ M).T with M = Wq Wk^T folded on the host ----
        qt3 = p_qt.tile([PD, KD, SH], F8, tag="qt", name="qt3")
        for m in range(KD):
            for n in range(SH // NT):
                ps = p_mm.tile([PD, NT], F32, tag="mm", name=f"ps_q{m}_{n}")
                for k in range(0, KD, 2):
                    nc.tensor.matmul(
                        ps[:],
                        m3[:, k:k + 2, m * PD:(m + 1) * PD],
                        seqq3[:, k:k + 2, n * NT:(n + 1) * NT],
                        start=(k == 0),
                        stop=(k == KD - 2),
                        perf_mode=DR,
                    )
                nc.vector.tensor_copy(qt3[:, m, n * NT:(n + 1) * NT], ps[:])

        # ---- peer halves of the exchanges, via dynamic-slot DMA slices.
        # peer_slot (0|1) is a per-core host input; the sync engine blocks
        # here until each AllGather lands, but everything else it must issue
        # before the O phase (inputs, ib_v bounces, sh3) is already in flight.
        preg = nc.sync.alloc_register("peer_slot")
        nc.sync.reg_load(preg, peer_t[0:1, 0:1])
        pslot = nc.sync.snap(preg, donate=True, min_val=0, max_val=1)
        # batched gathers: one rearranged DMA per collective output rather
        # than one per 128-row chunk (each sync issue costs ~0.6us, paid
        # right after the collective completes -- on the critical path)
        seqp3 = p_sp.tile([PD, KD, SH], F8, tag="sp", name="seqp3")
        nc.sync.dma_start(
            seqp3[:],
            ob_x[bass.ds(pslot, 1), :, :].rearrange(
                "o (j p) s -> (o p) j s", p=PD),
        )
        v_other = p_vp.tile([PD, KH, D], F8, tag="vp", name="v_other")
        nc.sync.dma_start(
            v_other[:, 0:KH // 2, :],
            ob_v1[bass.ds(pslot, 1), :, :].rearrange(
                "o (j p) d -> (o p) j d", p=PD),
        )
        nc.sync.dma_start(
            v_other[:, KH // 2:KH, :],
            ob_v2[bass.ds(pslot, 1), :, :].rearrange(
                "o (j p) d -> (o p) j d", p=PD),
        )

        # ---- scoresT -> exp(shifted) -> colsum, local-first key order ------
        # 32.0 (exact in fp8) folds V's W_SCALE into the colsum so the
        # reciprocal needs no extra rescale
        ones3 = p_msc.tile([PD, 2, 16], F8, tag="ones", name="ones3")
        nc.vector.memset(ones3[:], float(W_SCALE))
        ebias = p_msc.tile([PD, 1], F32, tag="ebias", name="ebias")
        nc.vector.memset(ebias[:], EXP_SHIFT)
        cs_ps = p_cs.tile([1, SH], F32, tag="cs", name="cs")
        at3 = p_at.tile([PD, KC, SH], F8, tag="at", name="at3")

        def colsum_mm(m):
            for n in range(SH // NT):
                nc.tensor.matmul(
                    cs_ps[:, n * NT:(n + 1) * NT],
                    ones3[:, 0:2, 0:1],
                    at3[:, m:m + 2, n * NT:(n + 1) * NT],
                    start=(m == 0),
                    stop=(m == KC - 2),
                    perf_mode=DR,
                )

        # key chunks 0..7 score against seqq3 (own, no exchange wait),
        # 8..15 against the peer half; the AllGather hides under 0..7.
        for m in range(KC):
            sq3 = seqq3 if m < KD else seqp3
            mm_ = m if m < KD else m - KD
            for n in range(SH // NT):
                ps = p_mm.tile([PD, NT], F32, tag="mm", name=f"ps_s{m}_{n}")
                for k in range(0, KD, 2):
                    nc.tensor.matmul(
                        ps[:],
                        sq3[:, k:k + 2, mm_ * PD:(mm_ + 1) * PD],
                        qt3[:, k:k + 2, n * NT:(n + 1) * NT],
                        start=(k == 0),
                        stop=(k == KD - 2),
                        perf_mode=DR,
                    )
                nc.scalar.activation(
                    at3[:, m, n * NT:(n + 1) * NT], ps[:], Exp,
                    bias=ebias[:], scale=EXP_SCALE,
                )
            # colsum pairs ksubs (m, m+1); emit one pair late so the PE
            # never waits on ACT's exp
            if m >= 3 and m % 2 == 1:
                colsum_mm(m - 3)
        colsum_mm(KC - 2)

        # ---- 1/(32*colsum) transposed to per-partition [128, 8] -------------
        # reciprocal on [1, SH] runs on one DVE lane (6.5us); transpose the
        # colsum to [128, 8] via a DRAM bounce first so it takes ~0.2us.
        cs_sb = p_msc.tile([1, SH], F32, tag="cs_sb", name="cs_sb")
        nc.vector.tensor_copy(cs_sb[:], cs_ps[:])
        cs_d = p_dram.tile([1, SH], F32, tag="csd", name="cs_d")
        nc.gpsimd.dma_start(cs_d[:], cs_sb[:])
        csT = p_msc.tile([PD, KH], F32, tag="csT", name="csT")
        nc.gpsimd.dma_start(csT[:, :], cs_d.rearrange("o (m p) -> (o p) m", p=PD))
        recipT = p_msc.tile([PD, KH], F32, tag="recipT", name="recipT")
        nc.vector.reciprocal(recipT[:, :], csT[:, :])

        # ---- O = AT.T @ V in [q, d] layout; fused normalize + residual ------
        # out[q, d] = (sum_key at[key, q] * v[key, d]) * recip[q] + seq[q, d]
        # key chunk pairs 0..7 stream from vo3 (own V), 8..15 from v_other.
        for m in range(KH):
            o_t = p_o.tile([PD, D], BF16, tag="o", name=f"o{m}")
            for n in range(D // NT):
                ps = p_mm.tile([PD, NT], F32, tag="mm", name=f"ps_o{m}_{n}")
                for k in range(0, KC, 2):
                    v3t = vo3 if k < KH else v_other
                    kk = k if k < KH else k - KH
                    nc.tensor.matmul(
                        ps[:],
                        at3[:, k:k + 2, m * PD:(m + 1) * PD],
                        v3t[:, kk:kk + 2, n * NT:(n + 1) * NT],
                        start=(k == 0),
                        stop=(k == KC - 2),
                        perf_mode=DR,
                    )
                nc.vector.scalar_tensor_tensor(
                    o_t[:, n * NT:(n + 1) * NT],
                    ps[:],
                    recipT[:, m:m + 1],
                    sh3[:, m, n * NT:(n + 1) * NT],
                    op0=mybir.AluOpType.mult,
                    op1=mybir.AluOpType.add,
                )
                if m == KH - 1:
                    # last tile: store each half as soon as its stt lands so
                    # the final store is 256KB, not 512KB, off the tail
                    nc.sync.dma_start(
                        outT[m * PD:(m + 1) * PD, n * NT:(n + 1) * NT],
                        o_t[:, n * NT:(n + 1) * NT])
            if m < KH - 1:
                nc.sync.dma_start(outT[m * PD:(m + 1) * PD, :], o_t[:])


_NC_CACHE = None


def _get_nc():
    global _NC_CACHE
    if _NC_CACHE is None:
        nc = bacc.Bacc(
            "TRN2", target_bir_lowering=False, debug=False, num_devices=N_CORES
        )
        with tile.TileContext(nc) as tc:
            _build_kernel(tc)
        nc.compile()
        _NC_CACHE = nc
    return _NC_CACHE


def _prep_in_maps(seq, Wq, Wk, Wv, mask):
    seq = np.asarray(seq, dtype=np.float32)
    M = np.asarray(Wq, dtype=np.float32) @ np.asarray(Wk, dtype=np.float32).T
    m_f8 = (M * W_SCALE).astype(_FP8)
    wvm_f8 = (np.asarray(Wv, dtype=np.float32)
              * np.asarray(mask, dtype=np.float32)[None, :] * W_SCALE).astype(_FP8)
    in_maps = []
    for c in range(N_CORES):
        b, h = divmod(c, 2)
        seqT_own = np.ascontiguousarray(seq[b, h * SH:(h + 1) * SH, :].T)  # [D, SH]
        in_maps.append({
            "seqTq": seqT_own.astype(_FP8),
            "m3w": m_f8,
            "wv": wvm_f8,
            "seqTh": np.ascontiguousarray(
                seq[b, h * SH:(h + 1) * SH, :]).astype(ml_dtypes.bfloat16),
            "peer": np.array([[1 - h]], dtype=np.uint32),
        })
    return in_maps


def _run(seq, Wq, Wk, Wv, mask, trace=False, **run_kwargs):
    nc = _get_nc()
    in_maps = _prep_in_maps(seq, Wq, Wk, Wv, mask)
    res = bass_utils.run_bass_kernel_spmd(
        nc, in_maps, core_ids=list(range(N_CORES)), trace=trace, **run_kwargs
    )
    out = np.empty((B, S, D), dtype=np.float32)
    for c in range(N_CORES):
        b, h = divmod(c, 2)
        out[b, h * SH:(h + 1) * SH, :] = np.asarray(
            res.results[c]["outT"]).astype(np.float32)
    return out, res


def kernel(seq, Wq, Wk, Wv, mask):
    out, _ = _run(seq, Wq, Wk, Wv, mask)
    return out


# revision 53
# speedup vs baseline: 1.1164x; 1.0330x over previous
"""Single-head full-attention layer on 8 Trainium2 NeuronCores (fp8 DoubleRow).

reference:
    q = seq @ Wq; k = seq @ Wk; v = seq @ Wv          # [B,S,D], D=1024
    scores = q @ k.T / sqrt(D)                        # [B,S,S]
    out = seq + softmax(scores) @ v * mask            # [B,S,D]

Sharding: 8 cores = 4 batches x 2 sequence-halves, each core owning 1024
tokens.  The K projection never runs on device: the host folds
M = Wq @ Wk^T (scores = seq @ M @ seq_full^T), so the only cross-core
data needed for scores is the RAW transposed input sequence -- which is
an input tensor with zero compute dependency.  Its pair-AllGather is
issued as the kernel's first instruction and completes long before the
scores phase consumes the peer half.  V is projected locally for own
keys and exchanged via a second AllGather that hides under Q'/scores.

Attention runs in a per-core "local-first" key order [own 1024 keys,
peer 1024 keys] (softmax+PV are key-permutation invariant), keeping the
SPMD program identical on all cores; the peer halves of both AllGather
outputs are read with dynamic (register-offset) DMA slices, the slot
index coming from a per-core host input.

All matmuls run in fp8(E4M3) with perf_mode=DoubleRow (two contraction
rows per PE cell): operands live in 3D SBUF tiles [128, ksub, free] and
each matmul consumes a [:, k:k+2, :] slice. Numerics:
  - host scales M and Wv by 32 before the fp8 cast; seq is cast raw
    (N(0,1) fits fp8).  Q' = seq @ M lands at 32x, exactly like the old
    q, so exp's scale is 2^-10 (1/sqrt(D) * 1/32).
  - exp is shifted by -3 so attn values stay below fp8e4's +-240 max
    (softmax is shift-invariant).
  - scores/colsum/out accumulate in fp32 PSUM; the last matmul emits
    the output in [q, d] layout so 1/colsum is a per-partition scalar
    and normalize + residual-add (bf16 residual) fuse into one
    scalar_tensor_tensor.  The output mask is folded into Wv on the
    host.  The colsum PSUM tile is [2, 512] (one bank) so the matmul
    pool can hold 7 banks.
"""

import numpy as np
import ml_dtypes

import concourse.bass as bass
import concourse.mybir as mybir
import concourse.tile as tile
from concourse import bacc, bass_utils

B, S, D = 4, 2048, 1024
N_CORES = 8
SH = S // 2          # queries / own keys per core
PD = 128             # partition dim
KD = D // PD         # 8 ksub chunks over d
KH = SH // PD        # 8 ksub chunks over own keys
KC = S // PD         # 16 ksub chunks over all keys
NT = 512             # matmul free-dim tile (one PSUM bank of fp32)
F8 = mybir.dt.float8e4
F32 = mybir.dt.float32
BF16 = mybir.dt.bfloat16
W_SCALE = 32.0
EXP_SCALE = 1.0 / (32.0 * W_SCALE)     # 1/sqrt(D) / W_SCALE
EXP_SHIFT = -3.0
DR = mybir.MatmulPerfMode.DoubleRow
WARMUP = 230

_FP8 = ml_dtypes.float8_e4m3
_GROUPS = [[0, 1], [2, 3], [4, 5], [6, 7]]


def _build_kernel(tc):
    nc = tc.nc
    seqTq = nc.dram_tensor("seqTq", [D, SH], F8, kind="ExternalInput").ap()
    m3w = nc.dram_tensor("m3w", [D, D], F8, kind="ExternalInput").ap()
    wv = nc.dram_tensor("wv", [D, D], F8, kind="ExternalInput").ap()
    seqTh = nc.dram_tensor("seqTh", [SH, D], BF16, kind="ExternalInput").ap()
    peer_t = nc.dram_tensor("peer", [1, 1], mybir.dt.uint32, kind="ExternalInput")
    outT = nc.dram_tensor("outT", [SH, D], BF16, kind="ExternalOutput").ap()

    Exp = mybir.ActivationFunctionType.Exp

    with (
        tc.tile_pool(name="p_seq", bufs=1) as p_seq,
        tc.tile_pool(name="p_sp", bufs=1) as p_sp,
        tc.tile_pool(name="p_w", bufs=2) as p_w,
        tc.tile_pool(name="p_qt", bufs=1) as p_qt,
        tc.tile_pool(name="p_vo", bufs=1) as p_vo,
        tc.tile_pool(name="p_vp", bufs=1) as p_vp,
        tc.tile_pool(name="p_at", bufs=1) as p_at,
        tc.tile_pool(name="p_sh", bufs=1) as p_sh,
        tc.tile_pool(name="p_o", bufs=4) as p_o,
        tc.tile_pool(name="p_msc", bufs=1) as p_msc,
        tc.tile_pool(name="p_dram", bufs=1, space="DRAM") as p_dram,
        tc.tile_pool(name="p_mm", bufs=6, space="PSUM") as p_mm,
        tc.tile_pool(name="p_cs", bufs=1, space="PSUM") as p_cs,
    ):
        # ---- seq exchange: pure input data, no compute dependency -> the
        # AllGather is gated only by one DRAM->DRAM bounce of the input
        # (collectives cannot read IO tensors) and its latency hides under
        # warmup + V_own + Q' + scores-own entirely.
        ib_x = p_dram.tile([D, SH], F8, tag="ibx", name="ib_x")
        ob_x = p_dram.tile([2, D, SH], F8, tag="obx", name="ob_x")

        # ---- resident inputs.  The V phase's k-loop consumes (seqq, wv)
        # j-chunks in ascending order, so loading each tensor as two
        # interleaved half-chunks lets the first V matmuls start once 1MB
        # has landed instead of waiting for the full 2MB.  The collective
        # input bounce follows them -- its only deadline is the ~40us ncfw
        # stream-init floor, which it beats by a mile.
        HD = D // 2
        seqq3 = p_seq.tile([PD, KD, SH], F8, tag="seqq", name="seqq3")
        wv3 = p_w.tile([PD, KD, D], F8, tag="wv", name="wv3")
        nc.sync.dma_start(
            seqq3[:, 0:KD // 2, :],
            seqTq[0:HD, :].rearrange("(j p) s -> p j s", p=PD))
        nc.sync.dma_start(
            wv3[:, 0:KD // 2, :],
            wv[0:HD, :].rearrange("(j p) d -> p j d", p=PD))
        nc.sync.dma_start(
            seqq3[:, KD // 2:KD, :],
            seqTq[HD:D, :].rearrange("(j p) s -> p j s", p=PD))
        nc.sync.dma_start(
            wv3[:, KD // 2:KD, :],
            wv[HD:D, :].rearrange("(j p) d -> p j d", p=PD))
        nc.sync.dma_start(ib_x[:], seqTq)
        nc.gpsimd.collective_compute(
            "AllGather", mybir.AluOpType.bypass, replica_groups=_GROUPS,
            ins=[ib_x.opt()], outs=[ob_x.opt()],
        )
        m3 = p_w.tile([PD, KD, D], F8, tag="m3", name="m3")
        nc.sync.dma_start(m3[:], m3w.rearrange("(j p) d -> p j d", p=PD))
        # residual input (bf16)
        sh3 = p_sh.tile([PD, KH, D], BF16, tag="sh", name="sh3")
        nc.sync.dma_start(sh3[:], seqTh.rearrange("(j p) d -> p j d", p=PD))

        # V is exchanged as two half-size AllGathers so the first half's
        # data (queued on the cc stream behind the seq exchange) lands
        # before the O phase touches the peer V.
        ib_v = p_dram.tile([SH, D], F8, tag="ibv", name="ib_v")
        ob_v1 = p_dram.tile([2, SH // 2, D], F8, tag="obv1", name="ob_v1")
        ob_v2 = p_dram.tile([2, SH // 2, D], F8, tag="obv2", name="ob_v2")

        # ---- HAM warm-up: keep the PE busy on dummy matmuls during the
        # input-DMA wait so the clock gate is at 2.4GHz when real work starts
        wu_sb = p_msc.tile([PD, 2, 16], F8, tag="wu", name="wu_sb")
        nc.vector.memset(wu_sb[:], 0.0)
        ps_wu = p_mm.tile([PD, 16], F32, tag="mm", name="ps_wu")
        for i in range(WARMUP):
            nc.tensor.matmul(
                ps_wu[0:16, 0:16], wu_sb[:, 0:2, 0:16], wu_sb[:, 0:2, 0:16],
                start=(i == 0), stop=(i == WARMUP - 1), perf_mode=DR,
            )

        # ---- V_own = seq_own @ (Wv * mask), bounce out, AllGather ----------
        vo3 = p_vo.tile([PD, KH, D], F8, tag="vo", name="vo3")
        for m in range(KH):
            for n in range(D // NT):
                ps = p_mm.tile([PD, NT], F32, tag="mm", name=f"ps_v{m}_{n}")
                for k in range(0, KD, 2):
                    nc.tensor.matmul(
                        ps[:],
                        seqq3[:, k:k + 2, m * PD:(m + 1) * PD],
                        wv3[:, k:k + 2, n * NT:(n + 1) * NT],
                        start=(k == 0),
                        stop=(k == KD - 2),
                        perf_mode=DR,
                    )
                nc.vector.tensor_copy(vo3[:, m, n * NT:(n + 1) * NT], ps[:])
            nc.sync.dma_start(ib_v[m * PD:(m + 1) * PD, :], vo3[:, m, :])
            if m == KH // 2 - 1:
                nc.gpsimd.collective_compute(
                    "AllGather", mybir.AluOpType.bypass, replica_groups=_GROUPS,
                    ins=[ib_v[0:SH // 2, :]], outs=[ob_v1.opt()],
                )
        nc.gpsimd.collective_compute(
            "AllGather", mybir.AluOpType.bypass, replica_groups=_GROUPS,
            ins=[ib_v[SH // 2:SH, :]], outs=[ob_v2.opt()],
        )

        # ---- Q'^T = (seq_own @ M).T with M = Wq Wk^T folded on the host ----
        qt3 = p_qt.tile([PD, KD, SH], F8, tag="qt", name="qt3")
        for m in range(KD):
            for n in range(SH // NT):
                ps = p_mm.tile([PD, NT], F32, tag="mm", name=f"ps_q{m}_{n}")
                for k in range(0, KD, 2):
                    nc.tensor.matmul(
                        ps[:],
                        m3[:, k:k + 2, m * PD:(m + 1) * PD],
                        seqq3[:, k:k + 2, n * NT:(n + 1) * NT],
                        start=(k == 0),
                        stop=(k == KD - 2),
                        perf_mode=DR,
                    )
                nc.vector.tensor_copy(qt3[:, m, n * NT:(n + 1) * NT], ps[:])

        # ---- peer halves of the exchanges, via dynamic-slot DMA slices.
        # peer_slot (0|1) is a per-core host input; the sync engine blocks
        # here until each AllGather lands, but everything else it must issue
        # before the O phase (inputs, ib_v bounces, sh3) is already in flight.
        preg = nc.sync.alloc_register("peer_slot")
        nc.sync.reg_load(preg, peer_t[0:1, 0:1])
        pslot = nc.sync.snap(preg, donate=True, min_val=0, max_val=1)
        # batched gathers: one rearranged DMA per collective output rather
        # than one per 128-row chunk (each sync issue costs ~0.6us, paid
        # right after the collective completes -- on the critical path)
        seqp3 = p_sp.tile([PD, KD, SH], F8, tag="sp", name="seqp3")
        nc.sync.dma_start(
            seqp3[:],
            ob_x[bass.ds(pslot, 1), :, :].rearrange(
                "o (j p) s -> (o p) j s", p=PD),
        )
        v_other = p_vp.tile([PD, KH, D], F8, tag="vp", name="v_other")
        nc.sync.dma_start(
            v_other[:, 0:KH // 2, :],
            ob_v1[bass.ds(pslot, 1), :, :].rearrange(
                "o (j p) d -> (o p) j d", p=PD),
        )
        nc.sync.dma_start(
            v_other[:, KH // 2:KH, :],
            ob_v2[bass.ds(pslot, 1), :, :].rearrange(
                "o (j p) d -> (o p) j d", p=PD),
        )

        # ---- scoresT -> exp(shifted) -> colsum, local-first key order ------
        # 32.0 (exact in fp8) folds V's W_SCALE into the colsum so the
        # reciprocal needs no extra rescale
        ones3 = p_msc.tile([PD, 2, 16], F8, tag="ones", name="ones3")
        nc.vector.memset(ones3[:], float(W_SCALE))
        ebias = p_msc.tile([PD, 1], F32, tag="ebias", name="ebias")
        nc.vector.memset(ebias[:], EXP_SHIFT)
        cs_ps = p_cs.tile([1, SH], F32, tag="cs", name="cs")
        at3 = p_at.tile([PD, KC, SH], F8, tag="at", name="at3")

        def colsum_mm(m):
            for n in range(SH // NT):
                nc.tensor.matmul(
                    cs_ps[:, n * NT:(n + 1) * NT],
                    ones3[:, 0:2, 0:1],
                    at3[:, m:m + 2, n * NT:(n + 1) * NT],
                    start=(m == 0),
                    stop=(m == KC - 2),
                    perf_mode=DR,
                )

        # key chunks 0..7 score against seqq3 (own, no exchange wait),
        # 8..15 against the peer half; the AllGather hides under 0..7.
        for m in range(KC):
            sq3 = seqq3 if m < KD else seqp3
            mm_ = m if m < KD else m - KD
            for n in range(SH // NT):
                ps = p_mm.tile([PD, NT], F32, tag="mm", name=f"ps_s{m}_{n}")
                for k in range(0, KD, 2):
                    nc.tensor.matmul(
                        ps[:],
                        sq3[:, k:k + 2, mm_ * PD:(mm_ + 1) * PD],
                        qt3[:, k:k + 2, n * NT:(n + 1) * NT],
                        start=(k == 0),
                        stop=(k == KD - 2),
                        perf_mode=DR,
                    )
                nc.scalar.activation(
                    at3[:, m, n * NT:(n + 1) * NT], ps[:], Exp,
                    bias=ebias[:], scale=EXP_SCALE,
                )
            # colsum pairs ksubs (m, m+1); emit one pair late so the PE
            # never waits on ACT's exp
            if m >= 3 and m % 2 == 1:
                colsum_mm(m - 3)
        colsum_mm(KC - 2)

        # ---- 1/(32*colsum) transposed to per-partition [128, 8] -------------
        # reciprocal on [1, SH] runs on one DVE lane (6.5us); transpose the
        # colsum to [128, 8] via a DRAM bounce first so it takes ~0.2us.
        cs_sb = p_msc.tile([1, SH], F32, tag="cs_sb", name="cs_sb")
        nc.vector.tensor_copy(cs_sb[:], cs_ps[:])
        cs_d = p_dram.tile([1, SH], F32, tag="csd", name="cs_d")
        nc.gpsimd.dma_start(cs_d[:], cs_sb[:])
        csT = p_msc.tile([PD, KH], F32, tag="csT", name="csT")
        nc.gpsimd.dma_start(csT[:, :], cs_d.rearrange("o (m p) -> (o p) m", p=PD))
        recipT = p_msc.tile([PD, KH], F32, tag="recipT", name="recipT")
        nc.vector.reciprocal(recipT[:, :], csT[:, :])

        # ---- O = AT.T @ V in [q, d] layout; fused normalize + residual ------
        # out[q, d] = (sum_key at[key, q] * v[key, d]) * recip[q] + seq[q, d]
        # key chunk pairs 0..7 stream from vo3 (own V), 8..15 from v_other.
        def o_mm(ps, m, n, k, start, stop):
            v3t = vo3 if k < KH else v_other
            kk = k if k < KH else k - KH
            nc.tensor.matmul(
                ps[:],
                at3[:, k:k + 2, m * PD:(m + 1) * PD],
                v3t[:, kk:kk + 2, n * NT:(n + 1) * NT],
                start=start, stop=stop, perf_mode=DR,
            )

        def o_emit(o_t, ps, m, n):
            nc.vector.scalar_tensor_tensor(
                o_t[:, n * NT:(n + 1) * NT],
                ps[:],
                recipT[:, m:m + 1],
                sh3[:, m, n * NT:(n + 1) * NT],
                op0=mybir.AluOpType.mult,
                op1=mybir.AluOpType.add,
            )
            if m == KH - 1:
                # last tile: store each half as soon as its stt lands so
                # the final store is 256KB, not 512KB, off the tail
                nc.sync.dma_start(
                    outT[m * PD:(m + 1) * PD, n * NT:(n + 1) * NT],
                    o_t[:, n * NT:(n + 1) * NT])

        # The first two tiles hold their PSUM accumulation open across the
        # AG-V2-gated key chunks (12..15): both tiles' own+first-peer-half
        # work (4x12 matmuls) runs before the first v_other[4:8] read, so
        # the second V AllGather's completion latency hides under it.
        G = 2
        o_ts = [p_o.tile([PD, D], BF16, tag="o", name=f"o{m}") for m in range(G)]
        pss = {}
        for m in range(G):
            for n in range(D // NT):
                ps = p_mm.tile([PD, NT], F32, tag="mm", name=f"ps_o{m}_{n}")
                pss[(m, n)] = ps
                for k in range(0, 12, 2):
                    o_mm(ps, m, n, k, start=(k == 0), stop=False)
        for m in range(G):
            for n in range(D // NT):
                ps = pss[(m, n)]
                for k in range(12, KC, 2):
                    o_mm(ps, m, n, k, start=False, stop=(k == KC - 2))
                o_emit(o_ts[m], ps, m, n)
            nc.sync.dma_start(outT[m * PD:(m + 1) * PD, :], o_ts[m][:])
        for m in range(G, KH):
            o_t = p_o.tile([PD, D], BF16, tag="o", name=f"o{m}")
            for n in range(D // NT):
                ps = p_mm.tile([PD, NT], F32, tag="mm", name=f"ps_o{m}_{n}")
                for k in range(0, KC, 2):
                    o_mm(ps, m, n, k, start=(k == 0), stop=(k == KC - 2))
                o_emit(o_t, ps, m, n)
            if m < KH - 1:
                nc.sync.dma_start(outT[m * PD:(m + 1) * PD, :], o_t[:])


_NC_CACHE = None


def _get_nc():
    global _NC_CACHE
    if _NC_CACHE is None:
        nc = bacc.Bacc(
            "TRN2", target_bir_lowering=False, debug=False, num_devices=N_CORES
        )
        with tile.TileContext(nc) as tc:
            _build_kernel(tc)
        nc.compile()
        _NC_CACHE = nc
    return _NC_CACHE


def _prep_in_maps(seq, Wq, Wk, Wv, mask):
    seq = np.asarray(seq, dtype=np.float32)
    M = np.asarray(Wq, dtype=np.float32) @ np.asarray(Wk, dtype=np.float32).T
    m_f8 = (M * W_SCALE).astype(_FP8)
    wvm_f8 = (np.asarray(Wv, dtype=np.float32)
              * np.asarray(mask, dtype=np.float32)[None, :] * W_SCALE).astype(_FP8)
    in_maps = []
    for c in range(N_CORES):
        b, h = divmod(c, 2)
        seqT_own = np.ascontiguousarray(seq[b, h * SH:(h + 1) * SH, :].T)  # [D, SH]
        in_maps.append({
            "seqTq": seqT_own.astype(_FP8),
            "m3w": m_f8,
            "wv": wvm_f8,
            "seqTh": np.ascontiguousarray(
                seq[b, h * SH:(h + 1) * SH, :]).astype(ml_dtypes.bfloat16),
            "peer": np.array([[1 - h]], dtype=np.uint32),
        })
    return in_maps


def _run(seq, Wq, Wk, Wv, mask, trace=False, **run_kwargs):
    nc = _get_nc()
    in_maps = _prep_in_maps(seq, Wq, Wk, Wv, mask)
    res = bass_utils.run_bass_kernel_spmd(
        nc, in_maps, core_ids=list(range(N_CORES)), trace=trace, **run_kwargs
    )
    out = np.empty((B, S, D), dtype=np.float32)
    for c in range(N_CORES):
        b, h = divmod(c, 2)
        out[b, h * SH:(h + 1) * SH, :] = np.asarray(
            res.results[c]["outT"]).astype(np.float32)
    return out, res


def kernel(seq, Wq, Wk, Wv, mask):
    out, _ = _run(seq, Wq, Wk, Wv, mask)
    return out
